# revision 15
# baseline (speedup 1.0000x reference)
"""Chamfer rate-distortion loss on 8 TRN2 NeuronCores.

Layout: 8 cores = 4 batches x 2 chamfer directions. Each core computes, for
its (batch, direction), per-point nearest-neighbor squared distances of 8192
query points X against 8192 reference points Y.

Device algorithm per core (gathered cell windows):
  - Host splits queries into a 4x4 grid of xy-quantile cells (512 each);
    references are assigned to every cell whose region, expanded by DELTA,
    contains them.  Queries z-sorted within cell; chunks of SUB=64.
  - For each chunk the host gathers a W=112-wide candidate window from the
    cell's z-sorted reference list into that chunk's contiguous 176-col
    input region (64 query cols + 112 window cols), so all matmul/reduce
    offsets are static and DMA column slices arrive in consumption order.
  - matmul trick (fp16 hi/lo split, K=11 rows, ~1e-5 abs precision):
    PSUM[m,p] = SCALE^2*(|y_p|^2 - 2 x_m.y_p) = SCALE^2*(D[m,p] - |x_m|^2).
    2 col-tiled M=64 matmuls per 128-partition block stream concurrently on
    the PE; PSUM tiles hold 16 blocks (slot stride 128 f32 = bank-aligned);
    one DVE reduce_min per tile -> [128, 16].  The DVE reduce is the
    steady-state pacer (~2.0us per tile); input DMA (495KB over the 11
    partition-row engines) overlaps the first ~half of compute.
  - Soundness per query q: every reference outside its window is at distance
    >= gap(q) = min(z-gap to excluded in-cell refs, margin to cell boundary
    + DELTA).  Host verifies d_cap(q) <= gap(q) (d_cap = Morton-KNN upper
    bound); failures (~600/core on expected data) are recomputed exactly on
    host against the full reference set.
"""

import os

import numpy as np

B, M, P = 4, 8192, 8192
ZAX = 2              # z-sort axis within cells
G = 4                # G x G xy quantile grid
DELTA = 0.05         # cell region expansion for reference assignment
SUB = 64             # queries per chunk (one M=64 col-tiled matmul)
W = 112              # candidate window width per chunk
WPAD = 128           # PSUM slot stride in f32 (bank-aligned)
NCH = M // SUB       # 128 chunks
NBLK = NCH // 2      # 64 blocks of 128 partitions (2 col-tiled chunks)
SLOTS = 16           # blocks per PSUM tile ([128, 16, 128] f32 = 4 banks)
NT = NBLK // SLOTS   # 4 PSUM tiles per core
KROWS = 11
CHW = SUB + W        # 176 cols per chunk (64 query + 112 window)
WR_P = KROWS
WR_C = NCH * CHW     # 22528 cols, chunk-major consumption order
SCALE = 32.0
LMBDA = 5.0

_CACHE = {}
LAST_RESULTS = None


def _build_bass():
    import concourse.tile as tile
    from concourse import bacc, mybir

    nc = bacc.Bacc(None, target_bir_lowering=False, debug=False)
    f32 = mybir.dt.float32
    f16 = mybir.dt.float16

    wr_d = nc.dram_tensor("wr", [WR_P, WR_C], f16, kind="ExternalInput")
    out_d = nc.dram_tensor("out", [128, NBLK], f32, kind="ExternalOutput")

    with tile.TileContext(nc) as tc:
        with (
            tc.tile_pool(name="const", bufs=1) as cpool,
            tc.tile_pool(name="outp", bufs=1) as opool,
            tc.tile_pool(name="psum", bufs=2, space="PSUM") as ppool,
        ):
            wr = cpool.tile([WR_P, WR_C], f16)
            # chunk u occupies cols [160*u, 160*u+160) (64 query + 96
            # window), so column slices arrive in chunk-consumption order;
            # a small first slice gets the PE started early, and triggers
            # spread across SP/Activation/Pool issue in parallel
            # tile 0 (first 16 chunks) as two parallel 1408-col slices so
            # the reduce chain starts early; the rest in even 2816-col
            # slices round-robined across the three trigger engines
            cuts = [0, 1408, 2816] + [2816 * j for j in range(2, 9)]
            assert cuts[-1] == WR_C
            engs = [nc.sync, nc.scalar, nc.gpsimd]
            for j in range(len(cuts) - 1):
                engs[j % 3].dma_start(wr[:, cuts[j]:cuts[j + 1]],
                                      wr_d[:, cuts[j]:cuts[j + 1]])
            outt = opool.tile([128, NBLK], f32)

            for k in range(NT):
                ps = ppool.tile([128, SLOTS, WPAD], f32, tag="ps")
                for g in range(SLOTS):
                    for s in range(2):
                        u = 2 * (SLOTS * k + g) + s   # global chunk index
                        nc.tensor.matmul(
                            ps[64 * s:64 * s + 64, g, 0:W],
                            wr[:, CHW * u:CHW * u + SUB],
                            wr[:, CHW * u + SUB:CHW * (u + 1)],
                            start=True, stop=True,
                            tile_position=(0, 64 * s),
                        )
                if k == 0:
                    # halves: the DVE chain starts as soon as the first 8
                    # blocks' matmuls land instead of waiting for all 16
                    nc.vector.tensor_reduce(outt[:, 0:SLOTS // 2],
                                            ps[:, 0:SLOTS // 2, 0:W],
                                            axis=mybir.AxisListType.X,
                                            op=mybir.AluOpType.min)
                    nc.vector.tensor_reduce(outt[:, SLOTS // 2:SLOTS],
                                            ps[:, SLOTS // 2:SLOTS, 0:W],
                                            axis=mybir.AxisListType.X,
                                            op=mybir.AluOpType.min)
                else:
                    nc.vector.tensor_reduce(outt[:, SLOTS * k:SLOTS * (k + 1)],
                                            ps[:, :, 0:W],
                                            axis=mybir.AxisListType.X,
                                            op=mybir.AluOpType.min)
                if k == NT - 2:
                    # bulk of the output store overlaps the last tile's
                    # matmuls+reduce; trigger from the otherwise-idle ACT
                    nc.scalar.dma_start(out_d[:, 0:SLOTS * (NT - 1)],
                                        outt[:, 0:SLOTS * (NT - 1)])

            nc.sync.dma_start(out_d[:, SLOTS * (NT - 1):],
                              outt[:, SLOTS * (NT - 1):])
    nc.compile()
    return nc


def _morton_key(pts):
    rng = pts.max(0) - pts.min(0)
    q = ((pts - pts.min(0)) / (rng + 1e-9) * 1023).astype(np.uint64)

    def spread(x):
        x = x & np.uint64(0x3FF)
        x = (x | (x << np.uint64(16))) & np.uint64(0x30000FF)
        x = (x | (x << np.uint64(8))) & np.uint64(0x300F00F)
        x = (x | (x << np.uint64(4))) & np.uint64(0x30C30C3)
        x = (x | (x << np.uint64(2))) & np.uint64(0x9249249)
        return x

    return (spread(q[:, 0]) | (spread(q[:, 1]) << np.uint64(1))
            | (spread(q[:, 2]) << np.uint64(2)))


def _dcap2(X, Y, K=24):
    """Upper bound on squared NN distance via Morton-order neighbors."""
    allpts = np.concatenate([X, Y])
    mk = _morton_key(allpts)
    inv = np.empty(len(allpts), dtype=np.int64)
    inv[np.argsort(mk, kind="stable")] = np.arange(len(allpts))
    y_rank = inv[len(X):]
    order_y = np.argsort(y_rank, kind="stable")
    sorted_ranks = y_rank[order_y]
    idx = np.searchsorted(sorted_ranks, inv[:len(X)])
    cand = np.clip(idx[:, None] + np.arange(-K, K)[None, :], 0, len(Y) - 1)
    cands = order_y[cand]
    d2 = ((X[:, None, :] - Y[cands]) ** 2).sum(-1)
    return d2.min(1)


def _fp16_rows(V):
    """11-row fp16 hi/lo decomposition pieces for SCALE*V, V [N,3] f64.

    Returns (na, nb) for the query side and (c, e, wh, wl) for the
    reference side; device computes SCALE^2*(|y|^2 - 2 x.y) in fp32 PSUM:
      rows 0-2: na*c   rows 3-5: na*e   rows 6-8: nb*c   rows 9,10: 1*(wh,wl)
    """
    Vs = SCALE * V
    a = Vs.astype(np.float16)
    b = (Vs - a.astype(np.float64)).astype(np.float16)
    na = (-2.0 * a.astype(np.float64)).astype(np.float16)
    nb = (-2.0 * b.astype(np.float64)).astype(np.float16)
    w = (Vs ** 2).sum(1)
    wh = w.astype(np.float16)
    wl = (w - wh.astype(np.float64)).astype(np.float16)
    return na, nb, a, b, wh, wl


def _prep_core(X, Y):
    """Host prep for one (batch, direction)."""
    X = X.astype(np.float64)
    Y = Y.astype(np.float64)
    d_cap2 = _dcap2(X, Y)
    d_cap = np.sqrt(d_cap2)

    # 4x4 xy quantile cells over X
    nq = M // (G * G)
    o0 = np.argsort(X[:, 0], kind="stable")
    q_order = np.empty(M, dtype=np.int64)     # query index per sorted slot
    gap = np.empty(M)                          # soundness gap per slot
    widx = np.empty((NCH, W), dtype=np.int64)  # gathered ref indices per chunk
    pos = 0
    for i0 in range(G):
        strip = o0[i0 * (M // G):(i0 + 1) * (M // G)]
        lo0 = X[strip, 0].min() if i0 > 0 else -np.inf
        hi0 = X[strip, 0].max() if i0 < G - 1 else np.inf
        o1 = strip[np.argsort(X[strip, 1], kind="stable")]
        for i1 in range(G):
            cell = o1[i1 * nq:(i1 + 1) * nq]
            lo1 = X[cell, 1].min() if i1 > 0 else -np.inf
            hi1 = X[cell, 1].max() if i1 < G - 1 else np.inf
            sel = np.flatnonzero(
                (Y[:, 0] >= lo0 - DELTA) & (Y[:, 0] <= hi0 + DELTA)
                & (Y[:, 1] >= lo1 - DELTA) & (Y[:, 1] <= hi1 + DELTA))
            q = cell[np.argsort(X[cell, ZAX], kind="stable")]
            qz = X[q, ZAX]
            m_q = np.minimum.reduce([
                X[q, 0] - lo0, hi0 - X[q, 0],
                X[q, 1] - lo1, hi1 - X[q, 1]])
            gap_region = np.maximum(m_q, 0) + DELTA
            if len(sel) == 0:
                # no refs near this cell: every query is hard
                q_order[pos:pos + nq] = q
                gap[pos:pos + nq] = 0.0
                for u in range(nq // SUB):
                    widx[(pos // SUB) + u, :] = 0
                pos += nq
                continue
            ys = sel[np.argsort(Y[sel, ZAX], kind="stable")]
            yz = Y[ys, ZAX]
            ny = len(ys)
            for u in range(nq // SUB):
                sl = slice(u * SUB, (u + 1) * SUB)
                qzi = qz[sl]
                need = d_cap[q[sl]]
                salv = gap_region[sl] >= need
                if salv.any():
                    lo_t = np.searchsorted(yz, (qzi - need)[salv].min())
                    hi_t = np.searchsorted(yz, (qzi + need)[salv].max())
                else:
                    lo_t = np.searchsorted(yz, qzi[0])
                    hi_t = np.searchsorted(yz, qzi[-1])
                if hi_t - lo_t > W:
                    mid = (lo_t + hi_t) // 2
                    lo_p = max(0, mid - W // 2)
                else:
                    lo_p = max(0, lo_t - (W - (hi_t - lo_t)) // 2)
                lo_p = min(lo_p, max(0, ny - W))
                hi_p = min(ny, lo_p + W)
                ch = pos // SUB + u
                if ny >= W:
                    widx[ch, :] = ys[lo_p:lo_p + W]
                else:
                    widx[ch, :ny] = ys
                    widx[ch, ny:] = ys[ny - 1]
                gz = np.full(SUB, np.inf)
                if lo_p > 0:
                    gz = np.minimum(gz, qzi - yz[lo_p - 1])
                if hi_p < ny:
                    gz = np.minimum(gz, yz[hi_p] - qzi)
                gap[pos + u * SUB:pos + (u + 1) * SUB] = np.minimum(
                    np.maximum(gz, 0), gap_region[sl])
            q_order[pos:pos + nq] = q
            pos += nq

    hard = np.flatnonzero(~(d_cap[q_order] <= gap * 0.99))

    # fp16 hi/lo rows
    Xs = X[q_order]
    na, nb, _, _, _, _ = _fp16_rows(Xs)
    _, _, c, e, wh, wl = _fp16_rows(Y)

    wt = np.empty((KROWS, M), dtype=np.float16)
    naT, nbT = na.T, nb.T
    wt[0:3] = naT
    wt[3:6] = naT
    wt[6:9] = nbT
    wt[9:11] = 1.0

    wf = widx.reshape(-1)
    cT, eT = c.T, e.T
    rt = np.empty((KROWS, NCH * W), dtype=np.float16)
    rt[0:3] = cT[:, wf]
    rt[3:6] = eT[:, wf]
    rt[6:9] = cT[:, wf]
    rt[9] = wh[wf]
    rt[10] = wl[wf]

    wr = np.empty((WR_P, WR_C), dtype=np.float16)
    wr3 = wr.reshape(KROWS, NCH, CHW)
    wr3[:, :, 0:SUB] = wt.reshape(KROWS, NCH, SUB)
    wr3[:, :, SUB:] = rt.reshape(KROWS, NCH, W)

    X2 = (Xs ** 2).sum(1)
    return {"wr": wr}, {
        "q_order": q_order, "X2": X2, "hard": hard,
        "Xs": Xs, "Y": Y,
    }


def _post_core(out, meta):
    """Combine device output into sum over queries of min-D (float64)."""
    # out[p, blk]: query slot = blk*128 + p, chunk = slot block structure:
    # partition p = 32*s + j, block blk = k*16 + g, chunk u = 4*blk + s.
    p = np.arange(128)
    blk = np.arange(NBLK)
    s = p // 64
    j = p % 64
    slot = (2 * blk[None, :] + s[:, None]) * SUB + j[:, None]  # [128, NBLK]
    dev = np.full(M, np.inf)
    dev[slot.reshape(-1)] = out.reshape(-1).astype(np.float64)

    inv_s2 = 1.0 / (SCALE * SCALE)
    dmin = dev * inv_s2 + meta["X2"]

    hard = meta["hard"]
    if len(hard):
        Xh = meta["Xs"][hard]
        Y = meta["Y"]
        Y2 = (Y ** 2).sum(1)
        db = (Y2[None, :] - 2.0 * (Xh @ Y.T)).min(axis=1)
        dmin[hard] = db + meta["X2"][hard]
    return dmin.sum()


def _install_axon_profile_hook():
    """Make trace=True work under axon when the image's antenv lacks
    axon_hooks: inject a shim module wired to the ctypes NTFF driver."""
    import sys
    import types
    try:
        from antenv.axon_hooks import get_axon_ntff_profile_hook  # noqa: F401
        return
    except ImportError:
        pass
    try:
        import antenv
        from trn_agent_boot.trn_boot import _ntff_profile_via_ctypes
        hook = _ntff_profile_via_ctypes("/opt/axon/libaxon_pjrt.so")
    except Exception:
        hook = None
    mod = types.ModuleType("antenv.axon_hooks")
    state = {"h": hook}
    mod.get_axon_ntff_profile_hook = lambda: state["h"]
    mod.set_axon_ntff_profile_hook = lambda h: state.__setitem__("h", h)
    sys.modules["antenv.axon_hooks"] = mod
    try:
        antenv.axon_hooks = mod
    except Exception:
        pass


def kernel(x_hat, points, likelihoods):
    from concourse.bass_utils import run_bass_kernel_spmd
    global LAST_RESULTS

    trace = bool(int(os.environ.get("CHAMFER_TRACE", "0")))
    if trace:
        _install_axon_profile_hook()

    if "nc" not in _CACHE:
        _CACHE["nc"] = _build_bass()
    nc = _CACHE["nc"]

    in_maps, metas = [], []
    for core in range(8):
        b, d = core // 2, core % 2
        X = x_hat[b] if d == 0 else points[b]
        Y = points[b] if d == 0 else x_hat[b]
        m, meta = _prep_core(np.asarray(X), np.asarray(Y))
        in_maps.append(m)
        metas.append(meta)

    res = run_bass_kernel_spmd(
        nc, in_maps, core_ids=list(range(8)), trace=trace,
    )
    LAST_RESULTS = res

    sums = [_post_core(res.results[c]["out"], metas[c]) for c in range(8)]
    cham_x = sum(sums[c] for c in range(8) if c % 2 == 0) / (B * M)
    cham_y = sum(sums[c] for c in range(8) if c % 2 == 1) / (B * P)
    rec = cham_x + cham_y

    lik = np.asarray(likelihoods, dtype=np.float64)
    bpp = np.log2(lik).sum() / (-(B * P))

    loss = bpp + LMBDA * rec
    return np.array([loss, bpp, rec], dtype=np.float32)


# revision 16
# speedup vs baseline: 1.0270x; 1.0270x over previous
"""Chamfer rate-distortion loss on 8 TRN2 NeuronCores.

Layout: 8 cores = 4 batches x 2 chamfer directions. Each core computes, for
its (batch, direction), per-point nearest-neighbor squared distances of 8192
query points X against 8192 reference points Y.

Device algorithm per core (gathered cell windows):
  - Host splits queries into a 4x4 grid of xy-quantile cells (512 each);
    references are assigned to every cell whose region, expanded by DELTA,
    contains them.  Queries z-sorted within cell; chunks of SUB=64.
  - For each chunk the host gathers a W=112-wide candidate window from the
    cell's z-sorted reference list into that chunk's contiguous 176-col
    input region (64 query cols + 112 window cols), so all matmul/reduce
    offsets are static and DMA column slices arrive in consumption order.
  - matmul trick (fp16 hi/lo split, K=11 rows, ~1e-5 abs precision):
    PSUM[m,p] = SCALE^2*(|y_p|^2 - 2 x_m.y_p) = SCALE^2*(D[m,p] - |x_m|^2).
    2 col-tiled M=64 matmuls per 128-partition block stream concurrently on
    the PE; PSUM tiles hold 16 blocks (slot stride 128 f32 = bank-aligned);
    one DVE reduce_min per tile -> [128, 16].  The DVE reduce is the
    steady-state pacer (~2.0us per tile); input DMA (495KB over the 11
    partition-row engines) overlaps the first ~half of compute.
  - Soundness per query q: every reference outside its window is at distance
    >= gap(q) = min(z-gap to excluded in-cell refs, margin to cell boundary
    + DELTA).  Host verifies d_cap(q) <= gap(q) (d_cap = Morton-KNN upper
    bound); failures (~600/core on expected data) are recomputed exactly on
    host against the full reference set.
"""

import os

import numpy as np

B, M, P = 4, 8192, 8192
ZAX = 2              # z-sort axis within cells
G = 4                # G x G xy quantile grid
DELTA = 0.05         # cell region expansion for reference assignment
SUB = 64             # queries per chunk (one M=64 col-tiled matmul)
W = 112              # candidate window width per chunk
WPAD = 128           # PSUM slot stride in f32 (bank-aligned)
NCH = M // SUB       # 128 chunks
NBLK = NCH // 2      # 64 blocks of 128 partitions (2 col-tiled chunks)
SLOTS = 16           # blocks per PSUM tile ([128, 16, 128] f32 = 4 banks)
NT = NBLK // SLOTS   # 4 PSUM tiles per core
KROWS = 11
CHW = SUB + W        # 176 cols per chunk (64 query + 112 window)
WR_P = KROWS
WR_C = NCH * CHW     # 22528 cols, chunk-major consumption order
SCALE = 32.0
LMBDA = 5.0

_CACHE = {}
LAST_RESULTS = None


def _build_bass():
    import concourse.tile as tile
    from concourse import bacc, mybir

    nc = bacc.Bacc(None, target_bir_lowering=False, debug=False)
    f32 = mybir.dt.float32
    f16 = mybir.dt.float16

    wr_d = nc.dram_tensor("wr", [WR_P, WR_C], f16, kind="ExternalInput")
    out_d = nc.dram_tensor("out", [128, NBLK], f32, kind="ExternalOutput")

    with tile.TileContext(nc) as tc:
        with (
            tc.tile_pool(name="const", bufs=1) as cpool,
            tc.tile_pool(name="outp", bufs=1) as opool,
            tc.tile_pool(name="psum", bufs=2, space="PSUM") as ppool,
        ):
            wr = cpool.tile([WR_P, WR_C], f16)
            # chunk u occupies cols [160*u, 160*u+160) (64 query + 96
            # window), so column slices arrive in chunk-consumption order;
            # a small first slice gets the PE started early, and triggers
            # spread across SP/Activation/Pool issue in parallel
            SL = WR_C // 8
            cuts = [SL * j for j in range(9)]
            assert cuts[-1] == WR_C
            engs = [nc.sync, nc.scalar, nc.gpsimd]
            for j in range(len(cuts) - 1):
                engs[j % 3].dma_start(wr[:, cuts[j]:cuts[j + 1]],
                                      wr_d[:, cuts[j]:cuts[j + 1]])
            outt = opool.tile([128, NBLK], f32)

            for k in range(NT):
                ps = ppool.tile([128, SLOTS, WPAD], f32, tag="ps")
                for g in range(SLOTS):
                    for s in range(2):
                        u = 2 * (SLOTS * k + g) + s   # global chunk index
                        nc.tensor.matmul(
                            ps[64 * s:64 * s + 64, g, 0:W],
                            wr[:, CHW * u:CHW * u + SUB],
                            wr[:, CHW * u + SUB:CHW * (u + 1)],
                            start=True, stop=True,
                            tile_position=(0, 64 * s),
                        )
                if k == 0:
                    # halves: the DVE chain starts as soon as the first 8
                    # blocks' matmuls land instead of waiting for all 16
                    nc.vector.tensor_reduce(outt[:, 0:SLOTS // 2],
                                            ps[:, 0:SLOTS // 2, 0:W],
                                            axis=mybir.AxisListType.X,
                                            op=mybir.AluOpType.min)
                    nc.vector.tensor_reduce(outt[:, SLOTS // 2:SLOTS],
                                            ps[:, SLOTS // 2:SLOTS, 0:W],
                                            axis=mybir.AxisListType.X,
                                            op=mybir.AluOpType.min)
                else:
                    nc.vector.tensor_reduce(outt[:, SLOTS * k:SLOTS * (k + 1)],
                                            ps[:, :, 0:W],
                                            axis=mybir.AxisListType.X,
                                            op=mybir.AluOpType.min)
                if k == NT - 2:
                    # bulk of the output store overlaps the last tile's
                    # matmuls+reduce
                    nc.sync.dma_start(out_d[:, 0:SLOTS * (NT - 1)],
                                      outt[:, 0:SLOTS * (NT - 1)])

            nc.sync.dma_start(out_d[:, SLOTS * (NT - 1):],
                              outt[:, SLOTS * (NT - 1):])
    nc.compile()
    return nc


def _morton_key(pts):
    rng = pts.max(0) - pts.min(0)
    q = ((pts - pts.min(0)) / (rng + 1e-9) * 1023).astype(np.uint64)

    def spread(x):
        x = x & np.uint64(0x3FF)
        x = (x | (x << np.uint64(16))) & np.uint64(0x30000FF)
        x = (x | (x << np.uint64(8))) & np.uint64(0x300F00F)
        x = (x | (x << np.uint64(4))) & np.uint64(0x30C30C3)
        x = (x | (x << np.uint64(2))) & np.uint64(0x9249249)
        return x

    return (spread(q[:, 0]) | (spread(q[:, 1]) << np.uint64(1))
            | (spread(q[:, 2]) << np.uint64(2)))


def _dcap2(X, Y, K=24):
    """Upper bound on squared NN distance via Morton-order neighbors."""
    allpts = np.concatenate([X, Y])
    mk = _morton_key(allpts)
    inv = np.empty(len(allpts), dtype=np.int64)
    inv[np.argsort(mk, kind="stable")] = np.arange(len(allpts))
    y_rank = inv[len(X):]
    order_y = np.argsort(y_rank, kind="stable")
    sorted_ranks = y_rank[order_y]
    idx = np.searchsorted(sorted_ranks, inv[:len(X)])
    cand = np.clip(idx[:, None] + np.arange(-K, K)[None, :], 0, len(Y) - 1)
    cands = order_y[cand]
    d2 = ((X[:, None, :] - Y[cands]) ** 2).sum(-1)
    return d2.min(1)


def _fp16_rows(V):
    """11-row fp16 hi/lo decomposition pieces for SCALE*V, V [N,3] f64.

    Returns (na, nb) for the query side and (c, e, wh, wl) for the
    reference side; device computes SCALE^2*(|y|^2 - 2 x.y) in fp32 PSUM:
      rows 0-2: na*c   rows 3-5: na*e   rows 6-8: nb*c   rows 9,10: 1*(wh,wl)
    """
    Vs = SCALE * V
    a = Vs.astype(np.float16)
    b = (Vs - a.astype(np.float64)).astype(np.float16)
    na = (-2.0 * a.astype(np.float64)).astype(np.float16)
    nb = (-2.0 * b.astype(np.float64)).astype(np.float16)
    w = (Vs ** 2).sum(1)
    wh = w.astype(np.float16)
    wl = (w - wh.astype(np.float64)).astype(np.float16)
    return na, nb, a, b, wh, wl


def _prep_core(X, Y):
    """Host prep for one (batch, direction)."""
    X = X.astype(np.float64)
    Y = Y.astype(np.float64)
    d_cap2 = _dcap2(X, Y)
    d_cap = np.sqrt(d_cap2)

    # 4x4 xy quantile cells over X
    nq = M // (G * G)
    o0 = np.argsort(X[:, 0], kind="stable")
    q_order = np.empty(M, dtype=np.int64)     # query index per sorted slot
    gap = np.empty(M)                          # soundness gap per slot
    widx = np.empty((NCH, W), dtype=np.int64)  # gathered ref indices per chunk
    pos = 0
    for i0 in range(G):
        strip = o0[i0 * (M // G):(i0 + 1) * (M // G)]
        lo0 = X[strip, 0].min() if i0 > 0 else -np.inf
        hi0 = X[strip, 0].max() if i0 < G - 1 else np.inf
        o1 = strip[np.argsort(X[strip, 1], kind="stable")]
        for i1 in range(G):
            cell = o1[i1 * nq:(i1 + 1) * nq]
            lo1 = X[cell, 1].min() if i1 > 0 else -np.inf
            hi1 = X[cell, 1].max() if i1 < G - 1 else np.inf
            sel = np.flatnonzero(
                (Y[:, 0] >= lo0 - DELTA) & (Y[:, 0] <= hi0 + DELTA)
                & (Y[:, 1] >= lo1 - DELTA) & (Y[:, 1] <= hi1 + DELTA))
            q = cell[np.argsort(X[cell, ZAX], kind="stable")]
            qz = X[q, ZAX]
            m_q = np.minimum.reduce([
                X[q, 0] - lo0, hi0 - X[q, 0],
                X[q, 1] - lo1, hi1 - X[q, 1]])
            gap_region = np.maximum(m_q, 0) + DELTA
            if len(sel) == 0:
                # no refs near this cell: every query is hard
                q_order[pos:pos + nq] = q
                gap[pos:pos + nq] = 0.0
                for u in range(nq // SUB):
                    widx[(pos // SUB) + u, :] = 0
                pos += nq
                continue
            ys = sel[np.argsort(Y[sel, ZAX], kind="stable")]
            yz = Y[ys, ZAX]
            ny = len(ys)
            for u in range(nq // SUB):
                sl = slice(u * SUB, (u + 1) * SUB)
                qzi = qz[sl]
                need = d_cap[q[sl]]
                salv = gap_region[sl] >= need
                if salv.any():
                    lo_t = np.searchsorted(yz, (qzi - need)[salv].min())
                    hi_t = np.searchsorted(yz, (qzi + need)[salv].max())
                else:
                    lo_t = np.searchsorted(yz, qzi[0])
                    hi_t = np.searchsorted(yz, qzi[-1])
                if hi_t - lo_t > W:
                    mid = (lo_t + hi_t) // 2
                    lo_p = max(0, mid - W // 2)
                else:
                    lo_p = max(0, lo_t - (W - (hi_t - lo_t)) // 2)
                lo_p = min(lo_p, max(0, ny - W))
                hi_p = min(ny, lo_p + W)
                ch = pos // SUB + u
                if ny >= W:
                    widx[ch, :] = ys[lo_p:lo_p + W]
                else:
                    widx[ch, :ny] = ys
                    widx[ch, ny:] = ys[ny - 1]
                gz = np.full(SUB, np.inf)
                if lo_p > 0:
                    gz = np.minimum(gz, qzi - yz[lo_p - 1])
                if hi_p < ny:
                    gz = np.minimum(gz, yz[hi_p] - qzi)
                gap[pos + u * SUB:pos + (u + 1) * SUB] = np.minimum(
                    np.maximum(gz, 0), gap_region[sl])
            q_order[pos:pos + nq] = q
            pos += nq

    hard = np.flatnonzero(~(d_cap[q_order] <= gap * 0.99))

    # fp16 hi/lo rows
    Xs = X[q_order]
    na, nb, _, _, _, _ = _fp16_rows(Xs)
    _, _, c, e, wh, wl = _fp16_rows(Y)

    wt = np.empty((KROWS, M), dtype=np.float16)
    naT, nbT = na.T, nb.T
    wt[0:3] = naT
    wt[3:6] = naT
    wt[6:9] = nbT
    wt[9:11] = 1.0

    wf = widx.reshape(-1)
    cT, eT = c.T, e.T
    rt = np.empty((KROWS, NCH * W), dtype=np.float16)
    rt[0:3] = cT[:, wf]
    rt[3:6] = eT[:, wf]
    rt[6:9] = cT[:, wf]
    rt[9] = wh[wf]
    rt[10] = wl[wf]

    wr = np.empty((WR_P, WR_C), dtype=np.float16)
    wr3 = wr.reshape(KROWS, NCH, CHW)
    wr3[:, :, 0:SUB] = wt.reshape(KROWS, NCH, SUB)
    wr3[:, :, SUB:] = rt.reshape(KROWS, NCH, W)

    X2 = (Xs ** 2).sum(1)
    return {"wr": wr}, {
        "q_order": q_order, "X2": X2, "hard": hard,
        "Xs": Xs, "Y": Y,
    }


def _post_core(out, meta):
    """Combine device output into sum over queries of min-D (float64)."""
    # out[p, blk]: query slot = blk*128 + p, chunk = slot block structure:
    # partition p = 32*s + j, block blk = k*16 + g, chunk u = 4*blk + s.
    p = np.arange(128)
    blk = np.arange(NBLK)
    s = p // 64
    j = p % 64
    slot = (2 * blk[None, :] + s[:, None]) * SUB + j[:, None]  # [128, NBLK]
    dev = np.full(M, np.inf)
    dev[slot.reshape(-1)] = out.reshape(-1).astype(np.float64)

    inv_s2 = 1.0 / (SCALE * SCALE)
    dmin = dev * inv_s2 + meta["X2"]

    hard = meta["hard"]
    if len(hard):
        Xh = meta["Xs"][hard]
        Y = meta["Y"]
        Y2 = (Y ** 2).sum(1)
        db = (Y2[None, :] - 2.0 * (Xh @ Y.T)).min(axis=1)
        dmin[hard] = db + meta["X2"][hard]
    return dmin.sum()


def _install_axon_profile_hook():
    """Make trace=True work under axon when the image's antenv lacks
    axon_hooks: inject a shim module wired to the ctypes NTFF driver."""
    import sys
    import types
    try:
        from antenv.axon_hooks import get_axon_ntff_profile_hook  # noqa: F401
        return
    except ImportError:
        pass
    try:
        import antenv
        from trn_agent_boot.trn_boot import _ntff_profile_via_ctypes
        hook = _ntff_profile_via_ctypes("/opt/axon/libaxon_pjrt.so")
    except Exception:
        hook = None
    mod = types.ModuleType("antenv.axon_hooks")
    state = {"h": hook}
    mod.get_axon_ntff_profile_hook = lambda: state["h"]
    mod.set_axon_ntff_profile_hook = lambda h: state.__setitem__("h", h)
    sys.modules["antenv.axon_hooks"] = mod
    try:
        antenv.axon_hooks = mod
    except Exception:
        pass


def kernel(x_hat, points, likelihoods):
    from concourse.bass_utils import run_bass_kernel_spmd
    global LAST_RESULTS

    trace = bool(int(os.environ.get("CHAMFER_TRACE", "0")))
    if trace:
        _install_axon_profile_hook()

    if "nc" not in _CACHE:
        _CACHE["nc"] = _build_bass()
    nc = _CACHE["nc"]

    in_maps, metas = [], []
    for core in range(8):
        b, d = core // 2, core % 2
        X = x_hat[b] if d == 0 else points[b]
        Y = points[b] if d == 0 else x_hat[b]
        m, meta = _prep_core(np.asarray(X), np.asarray(Y))
        in_maps.append(m)
        metas.append(meta)

    res = run_bass_kernel_spmd(
        nc, in_maps, core_ids=list(range(8)), trace=trace,
    )
    LAST_RESULTS = res

    sums = [_post_core(res.results[c]["out"], metas[c]) for c in range(8)]
    cham_x = sum(sums[c] for c in range(8) if c % 2 == 0) / (B * M)
    cham_y = sum(sums[c] for c in range(8) if c % 2 == 1) / (B * P)
    rec = cham_x + cham_y

    lik = np.asarray(likelihoods, dtype=np.float64)
    bpp = np.log2(lik).sum() / (-(B * P))

    loss = bpp + LMBDA * rec
    return np.array([loss, bpp, rec], dtype=np.float32)


# revision 17
# speedup vs baseline: 1.1810x; 1.1500x over previous
"""Chamfer rate-distortion loss on 8 TRN2 NeuronCores.

Layout: 8 cores = 4 batches x 2 chamfer directions. Each core computes, for
its (batch, direction), per-point nearest-neighbor squared distances of 8192
query points X against 8192 reference points Y.

Device algorithm per core (gathered cell windows):
  - Host splits queries into a 4x4 grid of xy-quantile cells (512 each);
    references are assigned to every cell whose region, expanded by DELTA,
    contains them.  Queries z-sorted within cell; chunks of SUB=64.
  - For each chunk the host gathers a W=112-wide candidate window from the
    cell's z-sorted reference list into that chunk's contiguous 176-col
    input region (64 query cols + 112 window cols), so all matmul/reduce
    offsets are static and DMA column slices arrive in consumption order.
  - matmul trick (fp16 hi/lo split, K=11 rows, ~1e-5 abs precision):
    PSUM[m,p] = SCALE^2*(|y_p|^2 - 2 x_m.y_p) = SCALE^2*(D[m,p] - |x_m|^2).
    2 col-tiled M=64 matmuls per 128-partition block stream concurrently on
    the PE; PSUM tiles hold 16 blocks (slot stride 128 f32 = bank-aligned);
    one DVE reduce_min per tile -> [128, 16].  The DVE reduce is the
    steady-state pacer (~2.0us per tile); input DMA (495KB over the 11
    partition-row engines) overlaps the first ~half of compute.
  - Soundness per query q: every reference outside its window is at distance
    >= gap(q) = min(z-gap to excluded in-cell refs, margin to cell boundary
    + DELTA).  Host verifies d_cap(q) <= gap(q) (d_cap = Morton-KNN upper
    bound); failures (~600/core on expected data) are recomputed exactly on
    host against the full reference set.
"""

import os

import numpy as np

B, M, P = 4, 8192, 8192
ZAX = 2              # z-sort axis within cells
G = 4                # G x G xy quantile grid
DELTA = 0.05         # cell region expansion for reference assignment
SUB = 64             # queries per chunk (one M=64 col-tiled matmul)
W = 112              # candidate window width per chunk
WPAD = 128           # PSUM slot stride in f32 (bank-aligned)
NCH = M // SUB       # 128 chunks
NBLK = NCH // 2      # 64 blocks of 128 partitions (2 col-tiled chunks)
SLOTS = 16           # blocks per PSUM tile ([128, 16, 128] f32 = 4 banks)
NT = NBLK // SLOTS   # 4 PSUM tiles per core
KROWS = 11
CHW = SUB + W        # 176 cols per chunk (64 query + 112 window)
WR_P = KROWS
WR_C = NCH * CHW     # 22528 cols, chunk-major consumption order
SCALE = 32.0
LMBDA = 5.0

_CACHE = {}
LAST_RESULTS = None


def _build_bass():
    import concourse.tile as tile
    from concourse import bacc, mybir

    nc = bacc.Bacc(None, target_bir_lowering=False, debug=False)
    f32 = mybir.dt.float32
    f16 = mybir.dt.float16

    wr_d = nc.dram_tensor("wr", [WR_P, WR_C], f16, kind="ExternalInput")
    out_d = nc.dram_tensor("out", [128, NBLK], f32, kind="ExternalOutput")

    with tile.TileContext(nc) as tc:
        with (
            tc.tile_pool(name="const", bufs=1) as cpool,
            tc.tile_pool(name="outp", bufs=1) as opool,
            tc.tile_pool(name="psum", bufs=2, space="PSUM") as ppool,
        ):
            wr = cpool.tile([WR_P, WR_C], f16)
            # chunk u occupies cols [160*u, 160*u+160) (64 query + 96
            # window), so column slices arrive in chunk-consumption order;
            # a small first slice gets the PE started early, and triggers
            # spread across SP/Activation/Pool issue in parallel
            SL = WR_C // 8
            cuts = [SL * j for j in range(9)]
            assert cuts[-1] == WR_C
            engs = [nc.sync, nc.scalar, nc.gpsimd]
            for j in range(len(cuts) - 1):
                engs[j % 3].dma_start(wr[:, cuts[j]:cuts[j + 1]],
                                      wr_d[:, cuts[j]:cuts[j + 1]])
            outt = opool.tile([128, NBLK], f32)

            for k in range(NT):
                ps = ppool.tile([128, SLOTS, WPAD], f32, tag="ps")
                for g in range(SLOTS):
                    for s in range(2):
                        u = 2 * (SLOTS * k + g) + s   # global chunk index
                        nc.tensor.matmul(
                            ps[64 * s:64 * s + 64, g, 0:W],
                            wr[:, CHW * u:CHW * u + SUB],
                            wr[:, CHW * u + SUB:CHW * (u + 1)],
                            start=True, stop=True,
                            tile_position=(0, 64 * s),
                        )
                if k == 0:
                    # halves: the DVE chain starts as soon as the first 8
                    # blocks' matmuls land instead of waiting for all 16
                    nc.vector.tensor_reduce(outt[:, 0:SLOTS // 2],
                                            ps[:, 0:SLOTS // 2, 0:W],
                                            axis=mybir.AxisListType.X,
                                            op=mybir.AluOpType.min)
                    nc.vector.tensor_reduce(outt[:, SLOTS // 2:SLOTS],
                                            ps[:, SLOTS // 2:SLOTS, 0:W],
                                            axis=mybir.AxisListType.X,
                                            op=mybir.AluOpType.min)
                else:
                    nc.vector.tensor_reduce(outt[:, SLOTS * k:SLOTS * (k + 1)],
                                            ps[:, :, 0:W],
                                            axis=mybir.AxisListType.X,
                                            op=mybir.AluOpType.min)

            nc.sync.dma_start(out_d[:], outt[:])
    nc.compile()
    return nc


def _morton_key(pts):
    rng = pts.max(0) - pts.min(0)
    q = ((pts - pts.min(0)) / (rng + 1e-9) * 1023).astype(np.uint64)

    def spread(x):
        x = x & np.uint64(0x3FF)
        x = (x | (x << np.uint64(16))) & np.uint64(0x30000FF)
        x = (x | (x << np.uint64(8))) & np.uint64(0x300F00F)
        x = (x | (x << np.uint64(4))) & np.uint64(0x30C30C3)
        x = (x | (x << np.uint64(2))) & np.uint64(0x9249249)
        return x

    return (spread(q[:, 0]) | (spread(q[:, 1]) << np.uint64(1))
            | (spread(q[:, 2]) << np.uint64(2)))


def _dcap2(X, Y, K=24):
    """Upper bound on squared NN distance via Morton-order neighbors."""
    allpts = np.concatenate([X, Y])
    mk = _morton_key(allpts)
    inv = np.empty(len(allpts), dtype=np.int64)
    inv[np.argsort(mk, kind="stable")] = np.arange(len(allpts))
    y_rank = inv[len(X):]
    order_y = np.argsort(y_rank, kind="stable")
    sorted_ranks = y_rank[order_y]
    idx = np.searchsorted(sorted_ranks, inv[:len(X)])
    cand = np.clip(idx[:, None] + np.arange(-K, K)[None, :], 0, len(Y) - 1)
    cands = order_y[cand]
    d2 = ((X[:, None, :] - Y[cands]) ** 2).sum(-1)
    return d2.min(1)


def _fp16_rows(V):
    """11-row fp16 hi/lo decomposition pieces for SCALE*V, V [N,3] f64.

    Returns (na, nb) for the query side and (c, e, wh, wl) for the
    reference side; device computes SCALE^2*(|y|^2 - 2 x.y) in fp32 PSUM:
      rows 0-2: na*c   rows 3-5: na*e   rows 6-8: nb*c   rows 9,10: 1*(wh,wl)
    """
    Vs = SCALE * V
    a = Vs.astype(np.float16)
    b = (Vs - a.astype(np.float64)).astype(np.float16)
    na = (-2.0 * a.astype(np.float64)).astype(np.float16)
    nb = (-2.0 * b.astype(np.float64)).astype(np.float16)
    w = (Vs ** 2).sum(1)
    wh = w.astype(np.float16)
    wl = (w - wh.astype(np.float64)).astype(np.float16)
    return na, nb, a, b, wh, wl


def _prep_core(X, Y):
    """Host prep for one (batch, direction)."""
    X = X.astype(np.float64)
    Y = Y.astype(np.float64)
    d_cap2 = _dcap2(X, Y)
    d_cap = np.sqrt(d_cap2)

    # 4x4 xy quantile cells over X
    nq = M // (G * G)
    o0 = np.argsort(X[:, 0], kind="stable")
    q_order = np.empty(M, dtype=np.int64)     # query index per sorted slot
    gap = np.empty(M)                          # soundness gap per slot
    widx = np.empty((NCH, W), dtype=np.int64)  # gathered ref indices per chunk
    pos = 0
    for i0 in range(G):
        strip = o0[i0 * (M // G):(i0 + 1) * (M // G)]
        lo0 = X[strip, 0].min() if i0 > 0 else -np.inf
        hi0 = X[strip, 0].max() if i0 < G - 1 else np.inf
        o1 = strip[np.argsort(X[strip, 1], kind="stable")]
        for i1 in range(G):
            cell = o1[i1 * nq:(i1 + 1) * nq]
            lo1 = X[cell, 1].min() if i1 > 0 else -np.inf
            hi1 = X[cell, 1].max() if i1 < G - 1 else np.inf
            sel = np.flatnonzero(
                (Y[:, 0] >= lo0 - DELTA) & (Y[:, 0] <= hi0 + DELTA)
                & (Y[:, 1] >= lo1 - DELTA) & (Y[:, 1] <= hi1 + DELTA))
            q = cell[np.argsort(X[cell, ZAX], kind="stable")]
            qz = X[q, ZAX]
            m_q = np.minimum.reduce([
                X[q, 0] - lo0, hi0 - X[q, 0],
                X[q, 1] - lo1, hi1 - X[q, 1]])
            gap_region = np.maximum(m_q, 0) + DELTA
            if len(sel) == 0:
                # no refs near this cell: every query is hard
                q_order[pos:pos + nq] = q
                gap[pos:pos + nq] = 0.0
                for u in range(nq // SUB):
                    widx[(pos // SUB) + u, :] = 0
                pos += nq
                continue
            ys = sel[np.argsort(Y[sel, ZAX], kind="stable")]
            yz = Y[ys, ZAX]
            ny = len(ys)
            for u in range(nq // SUB):
                sl = slice(u * SUB, (u + 1) * SUB)
                qzi = qz[sl]
                need = d_cap[q[sl]]
                salv = gap_region[sl] >= need
                if salv.any():
                    lo_t = np.searchsorted(yz, (qzi - need)[salv].min())
                    hi_t = np.searchsorted(yz, (qzi + need)[salv].max())
                else:
                    lo_t = np.searchsorted(yz, qzi[0])
                    hi_t = np.searchsorted(yz, qzi[-1])
                if hi_t - lo_t > W:
                    mid = (lo_t + hi_t) // 2
                    lo_p = max(0, mid - W // 2)
                else:
                    lo_p = max(0, lo_t - (W - (hi_t - lo_t)) // 2)
                lo_p = min(lo_p, max(0, ny - W))
                hi_p = min(ny, lo_p + W)
                ch = pos // SUB + u
                if ny >= W:
                    widx[ch, :] = ys[lo_p:lo_p + W]
                else:
                    widx[ch, :ny] = ys
                    widx[ch, ny:] = ys[ny - 1]
                gz = np.full(SUB, np.inf)
                if lo_p > 0:
                    gz = np.minimum(gz, qzi - yz[lo_p - 1])
                if hi_p < ny:
                    gz = np.minimum(gz, yz[hi_p] - qzi)
                gap[pos + u * SUB:pos + (u + 1) * SUB] = np.minimum(
                    np.maximum(gz, 0), gap_region[sl])
            q_order[pos:pos + nq] = q
            pos += nq

    hard = np.flatnonzero(~(d_cap[q_order] <= gap * 0.99))

    # fp16 hi/lo rows
    Xs = X[q_order]
    na, nb, _, _, _, _ = _fp16_rows(Xs)
    _, _, c, e, wh, wl = _fp16_rows(Y)

    wt = np.empty((KROWS, M), dtype=np.float16)
    naT, nbT = na.T, nb.T
    wt[0:3] = naT
    wt[3:6] = naT
    wt[6:9] = nbT
    wt[9:11] = 1.0

    wf = widx.reshape(-1)
    cT, eT = c.T, e.T
    rt = np.empty((KROWS, NCH * W), dtype=np.float16)
    rt[0:3] = cT[:, wf]
    rt[3:6] = eT[:, wf]
    rt[6:9] = cT[:, wf]
    rt[9] = wh[wf]
    rt[10] = wl[wf]

    wr = np.empty((WR_P, WR_C), dtype=np.float16)
    wr3 = wr.reshape(KROWS, NCH, CHW)
    wr3[:, :, 0:SUB] = wt.reshape(KROWS, NCH, SUB)
    wr3[:, :, SUB:] = rt.reshape(KROWS, NCH, W)

    X2 = (Xs ** 2).sum(1)
    return {"wr": wr}, {
        "q_order": q_order, "X2": X2, "hard": hard,
        "Xs": Xs, "Y": Y,
    }


def _post_core(out, meta):
    """Combine device output into sum over queries of min-D (float64)."""
    # out[p, blk]: query slot = blk*128 + p, chunk = slot block structure:
    # partition p = 32*s + j, block blk = k*16 + g, chunk u = 4*blk + s.
    p = np.arange(128)
    blk = np.arange(NBLK)
    s = p // 64
    j = p % 64
    slot = (2 * blk[None, :] + s[:, None]) * SUB + j[:, None]  # [128, NBLK]
    dev = np.full(M, np.inf)
    dev[slot.reshape(-1)] = out.reshape(-1).astype(np.float64)

    inv_s2 = 1.0 / (SCALE * SCALE)
    dmin = dev * inv_s2 + meta["X2"]

    hard = meta["hard"]
    if len(hard):
        Xh = meta["Xs"][hard]
        Y = meta["Y"]
        Y2 = (Y ** 2).sum(1)
        db = (Y2[None, :] - 2.0 * (Xh @ Y.T)).min(axis=1)
        dmin[hard] = db + meta["X2"][hard]
    return dmin.sum()


def _install_axon_profile_hook():
    """Make trace=True work under axon when the image's antenv lacks
    axon_hooks: inject a shim module wired to the ctypes NTFF driver."""
    import sys
    import types
    try:
        from antenv.axon_hooks import get_axon_ntff_profile_hook  # noqa: F401
        return
    except ImportError:
        pass
    try:
        import antenv
        from trn_agent_boot.trn_boot import _ntff_profile_via_ctypes
        hook = _ntff_profile_via_ctypes("/opt/axon/libaxon_pjrt.so")
    except Exception:
        hook = None
    mod = types.ModuleType("antenv.axon_hooks")
    state = {"h": hook}
    mod.get_axon_ntff_profile_hook = lambda: state["h"]
    mod.set_axon_ntff_profile_hook = lambda h: state.__setitem__("h", h)
    sys.modules["antenv.axon_hooks"] = mod
    try:
        antenv.axon_hooks = mod
    except Exception:
        pass


def kernel(x_hat, points, likelihoods):
    from concourse.bass_utils import run_bass_kernel_spmd
    global LAST_RESULTS

    trace = bool(int(os.environ.get("CHAMFER_TRACE", "0")))
    if trace:
        _install_axon_profile_hook()

    if "nc" not in _CACHE:
        _CACHE["nc"] = _build_bass()
    nc = _CACHE["nc"]

    in_maps, metas = [], []
    for core in range(8):
        b, d = core // 2, core % 2
        X = x_hat[b] if d == 0 else points[b]
        Y = points[b] if d == 0 else x_hat[b]
        m, meta = _prep_core(np.asarray(X), np.asarray(Y))
        in_maps.append(m)
        metas.append(meta)

    res = run_bass_kernel_spmd(
        nc, in_maps, core_ids=list(range(8)), trace=trace,
    )
    LAST_RESULTS = res

    sums = [_post_core(res.results[c]["out"], metas[c]) for c in range(8)]
    cham_x = sum(sums[c] for c in range(8) if c % 2 == 0) / (B * M)
    cham_y = sum(sums[c] for c in range(8) if c % 2 == 1) / (B * P)
    rec = cham_x + cham_y

    lik = np.asarray(likelihoods, dtype=np.float64)
    bpp = np.log2(lik).sum() / (-(B * P))

    loss = bpp + LMBDA * rec
    return np.array([loss, bpp, rec], dtype=np.float32)


# revision 18
# speedup vs baseline: 1.2010x; 1.0169x over previous
"""Chamfer rate-distortion loss on 8 TRN2 NeuronCores.

Layout: 8 cores = 4 batches x 2 chamfer directions. Each core computes, for
its (batch, direction), per-point nearest-neighbor squared distances of 8192
query points X against 8192 reference points Y.

Device algorithm per core (gathered cell windows):
  - Host splits queries into a 4x4 grid of xy-quantile cells (512 each);
    references are assigned to every cell whose region, expanded by DELTA,
    contains them.  Queries z-sorted within cell; chunks of SUB=64.
  - For each chunk the host gathers a W=112-wide candidate window from the
    cell's z-sorted reference list into that chunk's contiguous 176-col
    input region (64 query cols + 112 window cols), so all matmul/reduce
    offsets are static and DMA column slices arrive in consumption order.
  - matmul trick (fp16 hi/lo split, K=11 rows, ~1e-5 abs precision):
    PSUM[m,p] = SCALE^2*(|y_p|^2 - 2 x_m.y_p) = SCALE^2*(D[m,p] - |x_m|^2).
    2 col-tiled M=64 matmuls per 128-partition block stream concurrently on
    the PE; PSUM tiles hold 16 blocks (slot stride 128 f32 = bank-aligned);
    one DVE reduce_min per tile -> [128, 16].  The DVE reduce is the
    steady-state pacer (~2.0us per tile); input DMA (495KB over the 11
    partition-row engines) overlaps the first ~half of compute.
  - Soundness per query q: every reference outside its window is at distance
    >= gap(q) = min(z-gap to excluded in-cell refs, margin to cell boundary
    + DELTA).  Host verifies d_cap(q) <= gap(q) (d_cap = Morton-KNN upper
    bound); failures (~600/core on expected data) are recomputed exactly on
    host against the full reference set.
"""

import os

import numpy as np

B, M, P = 4, 8192, 8192
ZAX = 2              # z-sort axis within cells
G = 4                # G x G xy quantile grid
DELTA = 0.04         # cell region expansion for reference assignment
SUB = 64             # queries per chunk (one M=64 col-tiled matmul)
W = 104              # candidate window width per chunk
WPAD = 128           # PSUM slot stride in f32 (bank-aligned)
NCH = M // SUB       # 128 chunks
NBLK = NCH // 2      # 64 blocks of 128 partitions (2 col-tiled chunks)
SLOTS = 16           # blocks per PSUM tile ([128, 16, 128] f32 = 4 banks)
NT = NBLK // SLOTS   # 4 PSUM tiles per core
KROWS = 11
CHW = SUB + W        # 168 cols per chunk (64 query + 104 window)
WR_P = KROWS
WR_C = NCH * CHW     # 21504 cols, chunk-major consumption order
SCALE = 32.0
LMBDA = 5.0

_CACHE = {}
LAST_RESULTS = None


def _build_bass():
    import concourse.tile as tile
    from concourse import bacc, mybir

    nc = bacc.Bacc(None, target_bir_lowering=False, debug=False)
    f32 = mybir.dt.float32
    f16 = mybir.dt.float16

    wr_d = nc.dram_tensor("wr", [WR_P, WR_C], f16, kind="ExternalInput")
    out_d = nc.dram_tensor("out", [128, NBLK], f32, kind="ExternalOutput")

    with tile.TileContext(nc) as tc:
        with (
            tc.tile_pool(name="const", bufs=1) as cpool,
            tc.tile_pool(name="outp", bufs=1) as opool,
            tc.tile_pool(name="psum", bufs=2, space="PSUM") as ppool,
        ):
            wr = cpool.tile([WR_P, WR_C], f16)
            # chunk u occupies cols [160*u, 160*u+160) (64 query + 96
            # window), so column slices arrive in chunk-consumption order;
            # a small first slice gets the PE started early, and triggers
            # spread across SP/Activation/Pool issue in parallel
            SL = WR_C // 8
            cuts = [SL * j for j in range(9)]
            assert cuts[-1] == WR_C
            engs = [nc.sync, nc.scalar, nc.gpsimd]
            for j in range(len(cuts) - 1):
                engs[j % 3].dma_start(wr[:, cuts[j]:cuts[j + 1]],
                                      wr_d[:, cuts[j]:cuts[j + 1]])
            outt = opool.tile([128, NBLK], f32)

            for k in range(NT):
                ps = ppool.tile([128, SLOTS, WPAD], f32, tag="ps")
                for g in range(SLOTS):
                    for s in range(2):
                        u = 2 * (SLOTS * k + g) + s   # global chunk index
                        nc.tensor.matmul(
                            ps[64 * s:64 * s + 64, g, 0:W],
                            wr[:, CHW * u:CHW * u + SUB],
                            wr[:, CHW * u + SUB:CHW * (u + 1)],
                            start=True, stop=True,
                            tile_position=(0, 64 * s),
                        )
                if k == 0:
                    # halves: the DVE chain starts as soon as the first 8
                    # blocks' matmuls land instead of waiting for all 16
                    nc.vector.tensor_reduce(outt[:, 0:SLOTS // 2],
                                            ps[:, 0:SLOTS // 2, 0:W],
                                            axis=mybir.AxisListType.X,
                                            op=mybir.AluOpType.min)
                    nc.vector.tensor_reduce(outt[:, SLOTS // 2:SLOTS],
                                            ps[:, SLOTS // 2:SLOTS, 0:W],
                                            axis=mybir.AxisListType.X,
                                            op=mybir.AluOpType.min)
                else:
                    nc.vector.tensor_reduce(outt[:, SLOTS * k:SLOTS * (k + 1)],
                                            ps[:, :, 0:W],
                                            axis=mybir.AxisListType.X,
                                            op=mybir.AluOpType.min)

            nc.sync.dma_start(out_d[:], outt[:])
    nc.compile()
    return nc


def _morton_key(pts):
    rng = pts.max(0) - pts.min(0)
    q = ((pts - pts.min(0)) / (rng + 1e-9) * 1023).astype(np.uint64)

    def spread(x):
        x = x & np.uint64(0x3FF)
        x = (x | (x << np.uint64(16))) & np.uint64(0x30000FF)
        x = (x | (x << np.uint64(8))) & np.uint64(0x300F00F)
        x = (x | (x << np.uint64(4))) & np.uint64(0x30C30C3)
        x = (x | (x << np.uint64(2))) & np.uint64(0x9249249)
        return x

    return (spread(q[:, 0]) | (spread(q[:, 1]) << np.uint64(1))
            | (spread(q[:, 2]) << np.uint64(2)))


def _dcap2(X, Y, K=24):
    """Upper bound on squared NN distance via Morton-order neighbors."""
    allpts = np.concatenate([X, Y])
    mk = _morton_key(allpts)
    inv = np.empty(len(allpts), dtype=np.int64)
    inv[np.argsort(mk, kind="stable")] = np.arange(len(allpts))
    y_rank = inv[len(X):]
    order_y = np.argsort(y_rank, kind="stable")
    sorted_ranks = y_rank[order_y]
    idx = np.searchsorted(sorted_ranks, inv[:len(X)])
    cand = np.clip(idx[:, None] + np.arange(-K, K)[None, :], 0, len(Y) - 1)
    cands = order_y[cand]
    d2 = ((X[:, None, :] - Y[cands]) ** 2).sum(-1)
    return d2.min(1)


def _fp16_rows(V):
    """11-row fp16 hi/lo decomposition pieces for SCALE*V, V [N,3] f64.

    Returns (na, nb) for the query side and (c, e, wh, wl) for the
    reference side; device computes SCALE^2*(|y|^2 - 2 x.y) in fp32 PSUM:
      rows 0-2: na*c   rows 3-5: na*e   rows 6-8: nb*c   rows 9,10: 1*(wh,wl)
    """
    Vs = SCALE * V
    a = Vs.astype(np.float16)
    b = (Vs - a.astype(np.float64)).astype(np.float16)
    na = (-2.0 * a.astype(np.float64)).astype(np.float16)
    nb = (-2.0 * b.astype(np.float64)).astype(np.float16)
    w = (Vs ** 2).sum(1)
    wh = w.astype(np.float16)
    wl = (w - wh.astype(np.float64)).astype(np.float16)
    return na, nb, a, b, wh, wl


def _prep_core(X, Y):
    """Host prep for one (batch, direction)."""
    X = X.astype(np.float64)
    Y = Y.astype(np.float64)
    d_cap2 = _dcap2(X, Y)
    d_cap = np.sqrt(d_cap2)

    # 4x4 xy quantile cells over X
    nq = M // (G * G)
    o0 = np.argsort(X[:, 0], kind="stable")
    q_order = np.empty(M, dtype=np.int64)     # query index per sorted slot
    gap = np.empty(M)                          # soundness gap per slot
    widx = np.empty((NCH, W), dtype=np.int64)  # gathered ref indices per chunk
    pos = 0
    for i0 in range(G):
        strip = o0[i0 * (M // G):(i0 + 1) * (M // G)]
        lo0 = X[strip, 0].min() if i0 > 0 else -np.inf
        hi0 = X[strip, 0].max() if i0 < G - 1 else np.inf
        o1 = strip[np.argsort(X[strip, 1], kind="stable")]
        for i1 in range(G):
            cell = o1[i1 * nq:(i1 + 1) * nq]
            lo1 = X[cell, 1].min() if i1 > 0 else -np.inf
            hi1 = X[cell, 1].max() if i1 < G - 1 else np.inf
            sel = np.flatnonzero(
                (Y[:, 0] >= lo0 - DELTA) & (Y[:, 0] <= hi0 + DELTA)
                & (Y[:, 1] >= lo1 - DELTA) & (Y[:, 1] <= hi1 + DELTA))
            q = cell[np.argsort(X[cell, ZAX], kind="stable")]
            qz = X[q, ZAX]
            m_q = np.minimum.reduce([
                X[q, 0] - lo0, hi0 - X[q, 0],
                X[q, 1] - lo1, hi1 - X[q, 1]])
            gap_region = np.maximum(m_q, 0) + DELTA
            if len(sel) == 0:
                # no refs near this cell: every query is hard
                q_order[pos:pos + nq] = q
                gap[pos:pos + nq] = 0.0
                for u in range(nq // SUB):
                    widx[(pos // SUB) + u, :] = 0
                pos += nq
                continue
            ys = sel[np.argsort(Y[sel, ZAX], kind="stable")]
            yz = Y[ys, ZAX]
            ny = len(ys)
            for u in range(nq // SUB):
                sl = slice(u * SUB, (u + 1) * SUB)
                qzi = qz[sl]
                need = d_cap[q[sl]]
                salv = gap_region[sl] >= need
                if salv.any():
                    lo_t = np.searchsorted(yz, (qzi - need)[salv].min())
                    hi_t = np.searchsorted(yz, (qzi + need)[salv].max())
                else:
                    lo_t = np.searchsorted(yz, qzi[0])
                    hi_t = np.searchsorted(yz, qzi[-1])
                if hi_t - lo_t > W:
                    mid = (lo_t + hi_t) // 2
                    lo_p = max(0, mid - W // 2)
                else:
                    lo_p = max(0, lo_t - (W - (hi_t - lo_t)) // 2)
                lo_p = min(lo_p, max(0, ny - W))
                hi_p = min(ny, lo_p + W)
                ch = pos // SUB + u
                if ny >= W:
                    widx[ch, :] = ys[lo_p:lo_p + W]
                else:
                    widx[ch, :ny] = ys
                    widx[ch, ny:] = ys[ny - 1]
                gz = np.full(SUB, np.inf)
                if lo_p > 0:
                    gz = np.minimum(gz, qzi - yz[lo_p - 1])
                if hi_p < ny:
                    gz = np.minimum(gz, yz[hi_p] - qzi)
                gap[pos + u * SUB:pos + (u + 1) * SUB] = np.minimum(
                    np.maximum(gz, 0), gap_region[sl])
            q_order[pos:pos + nq] = q
            pos += nq

    hard = np.flatnonzero(~(d_cap[q_order] <= gap * 0.99))

    # fp16 hi/lo rows
    Xs = X[q_order]
    na, nb, _, _, _, _ = _fp16_rows(Xs)
    _, _, c, e, wh, wl = _fp16_rows(Y)

    wt = np.empty((KROWS, M), dtype=np.float16)
    naT, nbT = na.T, nb.T
    wt[0:3] = naT
    wt[3:6] = naT
    wt[6:9] = nbT
    wt[9:11] = 1.0

    wf = widx.reshape(-1)
    cT, eT = c.T, e.T
    rt = np.empty((KROWS, NCH * W), dtype=np.float16)
    rt[0:3] = cT[:, wf]
    rt[3:6] = eT[:, wf]
    rt[6:9] = cT[:, wf]
    rt[9] = wh[wf]
    rt[10] = wl[wf]

    wr = np.empty((WR_P, WR_C), dtype=np.float16)
    wr3 = wr.reshape(KROWS, NCH, CHW)
    wr3[:, :, 0:SUB] = wt.reshape(KROWS, NCH, SUB)
    wr3[:, :, SUB:] = rt.reshape(KROWS, NCH, W)

    X2 = (Xs ** 2).sum(1)
    return {"wr": wr}, {
        "q_order": q_order, "X2": X2, "hard": hard,
        "Xs": Xs, "Y": Y,
    }


def _post_core(out, meta):
    """Combine device output into sum over queries of min-D (float64)."""
    # out[p, blk]: query slot = blk*128 + p, chunk = slot block structure:
    # partition p = 32*s + j, block blk = k*16 + g, chunk u = 4*blk + s.
    p = np.arange(128)
    blk = np.arange(NBLK)
    s = p // 64
    j = p % 64
    slot = (2 * blk[None, :] + s[:, None]) * SUB + j[:, None]  # [128, NBLK]
    dev = np.full(M, np.inf)
    dev[slot.reshape(-1)] = out.reshape(-1).astype(np.float64)

    inv_s2 = 1.0 / (SCALE * SCALE)
    dmin = dev * inv_s2 + meta["X2"]

    hard = meta["hard"]
    if len(hard):
        Xh = meta["Xs"][hard]
        Y = meta["Y"]
        Y2 = (Y ** 2).sum(1)
        db = (Y2[None, :] - 2.0 * (Xh @ Y.T)).min(axis=1)
        dmin[hard] = db + meta["X2"][hard]
    return dmin.sum()


def _install_axon_profile_hook():
    """Make trace=True work under axon when the image's antenv lacks
    axon_hooks: inject a shim module wired to the ctypes NTFF driver."""
    import sys
    import types
    try:
        from antenv.axon_hooks import get_axon_ntff_profile_hook  # noqa: F401
        return
    except ImportError:
        pass
    try:
        import antenv
        from trn_agent_boot.trn_boot import _ntff_profile_via_ctypes
        hook = _ntff_profile_via_ctypes("/opt/axon/libaxon_pjrt.so")
    except Exception:
        hook = None
    mod = types.ModuleType("antenv.axon_hooks")
    state = {"h": hook}
    mod.get_axon_ntff_profile_hook = lambda: state["h"]
    mod.set_axon_ntff_profile_hook = lambda h: state.__setitem__("h", h)
    sys.modules["antenv.axon_hooks"] = mod
    try:
        antenv.axon_hooks = mod
    except Exception:
        pass


def kernel(x_hat, points, likelihoods):
    from concourse.bass_utils import run_bass_kernel_spmd
    global LAST_RESULTS

    trace = bool(int(os.environ.get("CHAMFER_TRACE", "0")))
    if trace:
        _install_axon_profile_hook()

    if "nc" not in _CACHE:
        _CACHE["nc"] = _build_bass()
    nc = _CACHE["nc"]

    in_maps, metas = [], []
    for core in range(8):
        b, d = core // 2, core % 2
        X = x_hat[b] if d == 0 else points[b]
        Y = points[b] if d == 0 else x_hat[b]
        m, meta = _prep_core(np.asarray(X), np.asarray(Y))
        in_maps.append(m)
        metas.append(meta)

    res = run_bass_kernel_spmd(
        nc, in_maps, core_ids=list(range(8)), trace=trace,
    )
    LAST_RESULTS = res

    sums = [_post_core(res.results[c]["out"], metas[c]) for c in range(8)]
    cham_x = sum(sums[c] for c in range(8) if c % 2 == 0) / (B * M)
    cham_y = sum(sums[c] for c in range(8) if c % 2 == 1) / (B * P)
    rec = cham_x + cham_y

    lik = np.asarray(likelihoods, dtype=np.float64)
    bpp = np.log2(lik).sum() / (-(B * P))

    loss = bpp + LMBDA * rec
    return np.array([loss, bpp, rec], dtype=np.float32)


# revision 19
# speedup vs baseline: 1.2232x; 1.0185x over previous
"""Chamfer rate-distortion loss on 8 TRN2 NeuronCores.

Layout: 8 cores = 4 batches x 2 chamfer directions. Each core computes, for
its (batch, direction), per-point nearest-neighbor squared distances of 8192
query points X against 8192 reference points Y.

Device algorithm per core (gathered cell windows):
  - Host splits queries into a 4x4 grid of xy-quantile cells (512 each);
    references are assigned to every cell whose region, expanded by DELTA,
    contains them.  Queries z-sorted within cell; chunks of SUB=64.
  - For each chunk the host gathers a W=104-wide candidate window from the
    cell's z-sorted reference list into that chunk's contiguous 168-col
    input region (64 query cols + 104 window cols), so all matmul/reduce
    offsets are static and DMA column slices arrive in consumption order.
  - matmul trick (fp16 hi/lo split, K=11 rows, ~1e-5 abs precision):
    PSUM[m,p] = SCALE^2*(|y_p|^2 - 2 x_m.y_p) = SCALE^2*(D[m,p] - |x_m|^2).
    2 col-tiled M=64 matmuls per 128-partition block stream concurrently on
    the PE; PSUM tiles hold 16 blocks (slot stride 128 f32 = bank-aligned);
    one DVE reduce_min per tile -> [128, 16].  The DVE reduce is the
    steady-state pacer (~1.9us per tile); input DMA (462KB over the 11
    partition-row engines) overlaps the first ~half of compute.
  - Soundness per query q: every reference outside its window is at distance
    >= gap(q) = min(z-gap to excluded in-cell refs, margin to cell boundary
    + DELTA).  Host verifies d_cap(q) <= gap(q) (d_cap = Morton-KNN upper
    bound); failures (~700/core on expected data) are recomputed exactly on
    host against the full reference set.
"""

import os

import numpy as np

B, M, P = 4, 8192, 8192
ZAX = 2              # z-sort axis within cells
G = 4                # G x G xy quantile grid
DELTA = 0.04         # cell region expansion for reference assignment
SUB = 64             # queries per chunk (one M=64 col-tiled matmul)
W = 104              # candidate window width per chunk
WPAD = 128           # PSUM slot stride in f32 (bank-aligned)
NCH = M // SUB       # 128 chunks
NBLK = NCH // 2      # 64 blocks of 128 partitions (2 col-tiled chunks)
SLOTS = 16           # blocks per PSUM tile ([128, 16, 128] f32 = 4 banks)
NT = NBLK // SLOTS   # 4 PSUM tiles per core
KROWS = 11
CHW = SUB + W        # 168 cols per chunk (64 query + 104 window)
WR_P = KROWS
WR_C = NCH * CHW     # 21504 cols, chunk-major consumption order
SCALE = 32.0
LMBDA = 5.0

_CACHE = {}
LAST_RESULTS = None


def _build_bass():
    import concourse.tile as tile
    from concourse import bacc, mybir

    nc = bacc.Bacc(None, target_bir_lowering=False, debug=False)
    f32 = mybir.dt.float32
    f16 = mybir.dt.float16

    wr_d = nc.dram_tensor("wr", [WR_P, WR_C], f16, kind="ExternalInput")
    out_d = nc.dram_tensor("out", [128, NBLK], f32, kind="ExternalOutput")

    with tile.TileContext(nc) as tc:
        with (
            tc.tile_pool(name="const", bufs=1) as cpool,
            tc.tile_pool(name="outp", bufs=1) as opool,
            tc.tile_pool(name="psum", bufs=2, space="PSUM") as ppool,
        ):
            wr = cpool.tile([WR_P, WR_C], f16)
            # chunk u occupies cols [168*u, 168*u+168) (64 query + 104
            # window), so even column slices arrive in chunk-consumption
            # order; triggers spread across SP/Activation/Pool in parallel
            SL = WR_C // 8
            cuts = [SL * j for j in range(9)]
            assert cuts[-1] == WR_C
            engs = [nc.sync, nc.scalar, nc.gpsimd]
            for j in range(len(cuts) - 1):
                engs[j % 3].dma_start(wr[:, cuts[j]:cuts[j + 1]],
                                      wr_d[:, cuts[j]:cuts[j + 1]])
            outt = opool.tile([128, NBLK], f32)

            for k in range(NT):
                ps = ppool.tile([128, SLOTS, WPAD], f32, tag="ps")
                for g in range(SLOTS):
                    for s in range(2):
                        u = 2 * (SLOTS * k + g) + s   # global chunk index
                        nc.tensor.matmul(
                            ps[64 * s:64 * s + 64, g, 0:W],
                            wr[:, CHW * u:CHW * u + SUB],
                            wr[:, CHW * u + SUB:CHW * (u + 1)],
                            start=True, stop=True,
                            tile_position=(0, 64 * s),
                        )
                if k == 0:
                    # halves: the DVE chain starts as soon as the first 8
                    # blocks' matmuls land instead of waiting for all 16
                    nc.vector.tensor_reduce(outt[:, 0:SLOTS // 2],
                                            ps[:, 0:SLOTS // 2, 0:W],
                                            axis=mybir.AxisListType.X,
                                            op=mybir.AluOpType.min)
                    nc.vector.tensor_reduce(outt[:, SLOTS // 2:SLOTS],
                                            ps[:, SLOTS // 2:SLOTS, 0:W],
                                            axis=mybir.AxisListType.X,
                                            op=mybir.AluOpType.min)
                else:
                    nc.vector.tensor_reduce(outt[:, SLOTS * k:SLOTS * (k + 1)],
                                            ps[:, :, 0:W],
                                            axis=mybir.AxisListType.X,
                                            op=mybir.AluOpType.min)

            nc.sync.dma_start(out_d[:], outt[:])
    nc.compile()
    return nc


def _morton_key(pts):
    rng = pts.max(0) - pts.min(0)
    q = ((pts - pts.min(0)) / (rng + 1e-9) * 1023).astype(np.uint64)

    def spread(x):
        x = x & np.uint64(0x3FF)
        x = (x | (x << np.uint64(16))) & np.uint64(0x30000FF)
        x = (x | (x << np.uint64(8))) & np.uint64(0x300F00F)
        x = (x | (x << np.uint64(4))) & np.uint64(0x30C30C3)
        x = (x | (x << np.uint64(2))) & np.uint64(0x9249249)
        return x

    return (spread(q[:, 0]) | (spread(q[:, 1]) << np.uint64(1))
            | (spread(q[:, 2]) << np.uint64(2)))


def _dcap2(X, Y, K=24):
    """Upper bound on squared NN distance via Morton-order neighbors."""
    allpts = np.concatenate([X, Y])
    mk = _morton_key(allpts)
    inv = np.empty(len(allpts), dtype=np.int64)
    inv[np.argsort(mk, kind="stable")] = np.arange(len(allpts))
    y_rank = inv[len(X):]
    order_y = np.argsort(y_rank, kind="stable")
    sorted_ranks = y_rank[order_y]
    idx = np.searchsorted(sorted_ranks, inv[:len(X)])
    cand = np.clip(idx[:, None] + np.arange(-K, K)[None, :], 0, len(Y) - 1)
    cands = order_y[cand]
    d2 = ((X[:, None, :] - Y[cands]) ** 2).sum(-1)
    return d2.min(1)


def _fp16_rows(V):
    """11-row fp16 hi/lo decomposition pieces for SCALE*V, V [N,3] f64.

    Returns (na, nb) for the query side and (c, e, wh, wl) for the
    reference side; device computes SCALE^2*(|y|^2 - 2 x.y) in fp32 PSUM:
      rows 0-2: na*c   rows 3-5: na*e   rows 6-8: nb*c   rows 9,10: 1*(wh,wl)
    """
    Vs = SCALE * V
    a = Vs.astype(np.float16)
    b = (Vs - a.astype(np.float64)).astype(np.float16)
    na = (-2.0 * a.astype(np.float64)).astype(np.float16)
    nb = (-2.0 * b.astype(np.float64)).astype(np.float16)
    w = (Vs ** 2).sum(1)
    wh = w.astype(np.float16)
    wl = (w - wh.astype(np.float64)).astype(np.float16)
    return na, nb, a, b, wh, wl


def _prep_core(X, Y):
    """Host prep for one (batch, direction)."""
    X = X.astype(np.float64)
    Y = Y.astype(np.float64)
    d_cap2 = _dcap2(X, Y)
    d_cap = np.sqrt(d_cap2)

    # 4x4 xy quantile cells over X
    nq = M // (G * G)
    o0 = np.argsort(X[:, 0], kind="stable")
    q_order = np.empty(M, dtype=np.int64)     # query index per sorted slot
    gap = np.empty(M)                          # soundness gap per slot
    widx = np.empty((NCH, W), dtype=np.int64)  # gathered ref indices per chunk
    pos = 0
    for i0 in range(G):
        strip = o0[i0 * (M // G):(i0 + 1) * (M // G)]
        lo0 = X[strip, 0].min() if i0 > 0 else -np.inf
        hi0 = X[strip, 0].max() if i0 < G - 1 else np.inf
        o1 = strip[np.argsort(X[strip, 1], kind="stable")]
        for i1 in range(G):
            cell = o1[i1 * nq:(i1 + 1) * nq]
            lo1 = X[cell, 1].min() if i1 > 0 else -np.inf
            hi1 = X[cell, 1].max() if i1 < G - 1 else np.inf
            sel = np.flatnonzero(
                (Y[:, 0] >= lo0 - DELTA) & (Y[:, 0] <= hi0 + DELTA)
                & (Y[:, 1] >= lo1 - DELTA) & (Y[:, 1] <= hi1 + DELTA))
            q = cell[np.argsort(X[cell, ZAX], kind="stable")]
            qz = X[q, ZAX]
            m_q = np.minimum.reduce([
                X[q, 0] - lo0, hi0 - X[q, 0],
                X[q, 1] - lo1, hi1 - X[q, 1]])
            gap_region = np.maximum(m_q, 0) + DELTA
            if len(sel) == 0:
                # no refs near this cell: every query is hard
                q_order[pos:pos + nq] = q
                gap[pos:pos + nq] = 0.0
                for u in range(nq // SUB):
                    widx[(pos // SUB) + u, :] = 0
                pos += nq
                continue
            ys = sel[np.argsort(Y[sel, ZAX], kind="stable")]
            yz = Y[ys, ZAX]
            ny = len(ys)
            for u in range(nq // SUB):
                sl = slice(u * SUB, (u + 1) * SUB)
                qzi = qz[sl]
                need = d_cap[q[sl]]
                salv = gap_region[sl] >= need
                if salv.any():
                    lo_t = np.searchsorted(yz, (qzi - need)[salv].min())
                    hi_t = np.searchsorted(yz, (qzi + need)[salv].max())
                else:
                    lo_t = np.searchsorted(yz, qzi[0])
                    hi_t = np.searchsorted(yz, qzi[-1])
                if hi_t - lo_t > W:
                    mid = (lo_t + hi_t) // 2
                    lo_p = max(0, mid - W // 2)
                else:
                    lo_p = max(0, lo_t - (W - (hi_t - lo_t)) // 2)
                lo_p = min(lo_p, max(0, ny - W))
                hi_p = min(ny, lo_p + W)
                ch = pos // SUB + u
                if ny >= W:
                    widx[ch, :] = ys[lo_p:lo_p + W]
                else:
                    widx[ch, :ny] = ys
                    widx[ch, ny:] = ys[ny - 1]
                gz = np.full(SUB, np.inf)
                if lo_p > 0:
                    gz = np.minimum(gz, qzi - yz[lo_p - 1])
                if hi_p < ny:
                    gz = np.minimum(gz, yz[hi_p] - qzi)
                gap[pos + u * SUB:pos + (u + 1) * SUB] = np.minimum(
                    np.maximum(gz, 0), gap_region[sl])
            q_order[pos:pos + nq] = q
            pos += nq

    hard = np.flatnonzero(~(d_cap[q_order] <= gap * 0.99))

    # fp16 hi/lo rows
    Xs = X[q_order]
    na, nb, _, _, _, _ = _fp16_rows(Xs)
    _, _, c, e, wh, wl = _fp16_rows(Y)

    wt = np.empty((KROWS, M), dtype=np.float16)
    naT, nbT = na.T, nb.T
    wt[0:3] = naT
    wt[3:6] = naT
    wt[6:9] = nbT
    wt[9:11] = 1.0

    wf = widx.reshape(-1)
    cT, eT = c.T, e.T
    rt = np.empty((KROWS, NCH * W), dtype=np.float16)
    rt[0:3] = cT[:, wf]
    rt[3:6] = eT[:, wf]
    rt[6:9] = cT[:, wf]
    rt[9] = wh[wf]
    rt[10] = wl[wf]

    wr = np.empty((WR_P, WR_C), dtype=np.float16)
    wr3 = wr.reshape(KROWS, NCH, CHW)
    wr3[:, :, 0:SUB] = wt.reshape(KROWS, NCH, SUB)
    wr3[:, :, SUB:] = rt.reshape(KROWS, NCH, W)

    X2 = (Xs ** 2).sum(1)
    return {"wr": wr}, {
        "q_order": q_order, "X2": X2, "hard": hard,
        "Xs": Xs, "Y": Y,
    }


def _post_core(out, meta):
    """Combine device output into sum over queries of min-D (float64)."""
    # out[p, blk]: query slot = blk*128 + p, chunk = slot block structure:
    # partition p = 32*s + j, block blk = k*16 + g, chunk u = 4*blk + s.
    p = np.arange(128)
    blk = np.arange(NBLK)
    s = p // 64
    j = p % 64
    slot = (2 * blk[None, :] + s[:, None]) * SUB + j[:, None]  # [128, NBLK]
    dev = np.full(M, np.inf)
    dev[slot.reshape(-1)] = out.reshape(-1).astype(np.float64)

    inv_s2 = 1.0 / (SCALE * SCALE)
    dmin = dev * inv_s2 + meta["X2"]

    hard = meta["hard"]
    if len(hard):
        Xh = meta["Xs"][hard]
        Y = meta["Y"]
        Y2 = (Y ** 2).sum(1)
        db = (Y2[None, :] - 2.0 * (Xh @ Y.T)).min(axis=1)
        dmin[hard] = db + meta["X2"][hard]
    return dmin.sum()


def _install_axon_profile_hook():
    """Make trace=True work under axon when the image's antenv lacks
    axon_hooks: inject a shim module wired to the ctypes NTFF driver."""
    import sys
    import types
    try:
        from antenv.axon_hooks import get_axon_ntff_profile_hook  # noqa: F401
        return
    except ImportError:
        pass
    try:
        import antenv
        from trn_agent_boot.trn_boot import _ntff_profile_via_ctypes
        hook = _ntff_profile_via_ctypes("/opt/axon/libaxon_pjrt.so")
    except Exception:
        hook = None
    mod = types.ModuleType("antenv.axon_hooks")
    state = {"h": hook}
    mod.get_axon_ntff_profile_hook = lambda: state["h"]
    mod.set_axon_ntff_profile_hook = lambda h: state.__setitem__("h", h)
    sys.modules["antenv.axon_hooks"] = mod
    try:
        antenv.axon_hooks = mod
    except Exception:
        pass


def kernel(x_hat, points, likelihoods):
    from concourse.bass_utils import run_bass_kernel_spmd
    global LAST_RESULTS

    trace = bool(int(os.environ.get("CHAMFER_TRACE", "0")))
    if trace:
        _install_axon_profile_hook()

    if "nc" not in _CACHE:
        _CACHE["nc"] = _build_bass()
    nc = _CACHE["nc"]

    in_maps, metas = [], []
    for core in range(8):
        b, d = core // 2, core % 2
        X = x_hat[b] if d == 0 else points[b]
        Y = points[b] if d == 0 else x_hat[b]
        m, meta = _prep_core(np.asarray(X), np.asarray(Y))
        in_maps.append(m)
        metas.append(meta)

    res = run_bass_kernel_spmd(
        nc, in_maps, core_ids=list(range(8)), trace=trace,
    )
    LAST_RESULTS = res

    sums = [_post_core(res.results[c]["out"], metas[c]) for c in range(8)]
    cham_x = sum(sums[c] for c in range(8) if c % 2 == 0) / (B * M)
    cham_y = sum(sums[c] for c in range(8) if c % 2 == 1) / (B * P)
    rec = cham_x + cham_y

    lik = np.asarray(likelihoods, dtype=np.float64)
    bpp = np.log2(lik).sum() / (-(B * P))

    loss = bpp + LMBDA * rec
    return np.array([loss, bpp, rec], dtype=np.float32)
